# revision 2
# baseline (speedup 1.0000x reference)
"""nn_ActorCopy kernel for 8 trn2 NeuronCores.

Batch=1 sequential seq2seq actor with copy mechanism. The sequential decode
loop is latency-bound; this implementation runs the full recurrence on the
neuron devices via jax/PJRT (single-program), which keeps the large gen_W
matmul on-device. Shapes are hardcoded per the problem spec.
"""
import numpy as np
import jax
import jax.numpy as jnp
from jax import lax
from functools import partial

VOCAB = 32000
EMBED = 1024
HIDDEN = 1024
ML = 64
L = 50
HH = HIDDEN // 2


def _lstm_cell(x, h, c, Wih, Whh, bih, bhh):
    g = x @ Wih.T + h @ Whh.T + bih + bhh
    i, f, gg, o = jnp.split(g, 4)
    i = jax.nn.sigmoid(i)
    f = jax.nn.sigmoid(f)
    gg = jnp.tanh(gg)
    o = jax.nn.sigmoid(o)
    c = f * c + i * gg
    return o * jnp.tanh(c), c


@partial(jax.jit, static_argnums=())
def _forward(allowed_mask, embedding, Wih_f, Whh_f, bih_f, bhh_f,
             Wih_b, Whh_b, bih_b, bhh_b, Wih_d, Whh_d, bih_d, bhh_d,
             attn_W, attn_b, gen_W, gen_b, copy_W, copy_b, x_tokens):
    xemb = embedding[x_tokens]

    def enc_step(carry, xe):
        hf, cf, hb, cb = carry
        hf, cf = _lstm_cell(xe, hf, cf, Wih_f, Whh_f, bih_f, bhh_f)
        hb, cb = _lstm_cell(xe, hb, cb, Wih_b, Whh_b, bih_b, bhh_b)
        return (hf, cf, hb, cb), jnp.concatenate([hf, hb])

    z = jnp.zeros((HH,), xemb.dtype)
    (hf, cf, hb, cb), enc = lax.scan(enc_step, (z, z, z, z), xemb)
    enc_outs = jnp.zeros((ML, HIDDEN), xemb.dtype).at[:L].set(enc)
    h0 = jnp.concatenate([hf, hb])
    c0 = jnp.concatenate([cf, cb])

    copy_enc = jnp.tanh(enc_outs @ copy_W.T + copy_b)
    pos = jnp.arange(ML)
    sent_pad = jnp.full((ML,), -1, jnp.int32).at[:L].set(x_tokens)
    sos_emb = embedding[0]

    def dec_step(carry, t):
        h, c, prev_probs, prev_word, dec_in = carry
        first = t == 0
        a = jnp.concatenate([dec_in, h])
        attw = jax.nn.softmax(a @ attn_W.T + attn_b)
        attentive = attw @ enc_outs
        pc = prev_probs[VOCAB:]
        m = ((pos >= 1) & (pos < L - 1) & (sent_pad != prev_word)).astype(pc.dtype)
        pc = pc * m
        s = pc.sum()
        pc = jnp.where(s > 0, pc / jnp.where(s > 0, s, 1.0), pc)
        selective = pc @ enc_outs
        zero = jnp.zeros_like(attentive)
        attentive = jnp.where(first, zero, attentive)
        selective = jnp.where(first, zero, selective)
        h, c = _lstm_cell(jnp.concatenate([dec_in, selective, attentive]),
                          h, c, Wih_d, Whh_d, bih_d, bhh_d)
        gen = h @ gen_W.T + gen_b
        copy = copy_enc @ h
        probs = jnp.concatenate([gen, copy])
        probs = jax.nn.softmax(probs)
        dist = probs * allowed_mask
        mx = dist.max()
        all_idx = jnp.arange(VOCAB + ML, dtype=jnp.int32)
        aidx = jnp.min(jnp.where(dist >= mx, all_idx, VOCAB + ML))
        is_voc = aidx < VOCAB
        src = sent_pad[jnp.clip(aidx - VOCAB, 0, L - 1)]
        action = jnp.where(is_voc, aidx, src).astype(jnp.int32)
        prob = dist[aidx] + jnp.where(
            is_voc, jnp.zeros((), dist.dtype), dist[jnp.clip(action, 0, VOCAB - 1)])
        new = (h, c, lax.stop_gradient(probs), action, embedding[action])
        return new, (h, prob, action)

    carry0 = (h0, c0, jnp.zeros((VOCAB + ML,), h0.dtype),
              jnp.array(-1, jnp.int32), sos_emb)
    _, (hs, probs, actions) = lax.scan(dec_step, carry0, jnp.arange(ML))
    states = jnp.concatenate([h0[None], hs])
    return states, probs, actions


def kernel(x_tokens, allowed_mask, embedding, Wih_f, Whh_f, bih_f, bhh_f,
           Wih_b, Whh_b, bih_b, bhh_b, Wih_d, Whh_d, bih_d, bhh_d,
           attn_W, attn_b, gen_W, gen_b, copy_W, copy_b):
    dev = jax.devices()[0]
    args = [allowed_mask, embedding, Wih_f, Whh_f, bih_f, bhh_f,
            Wih_b, Whh_b, bih_b, bhh_b, Wih_d, Whh_d, bih_d, bhh_d,
            attn_W, attn_b, gen_W, gen_b, copy_W, copy_b, x_tokens]
    args = [jax.device_put(np.asarray(a), dev) for a in args]
    states, probs, actions = _forward(*args)
    return (np.asarray(states), np.asarray(probs),
            np.asarray(actions).astype(np.int32))
